# revision 1
# baseline (speedup 1.0000x reference)
"""Bahdanau additive attention kernel for Trainium2 (8 NeuronCores, SPMD).

Problem: hidden [32,1024], encoder_outputs [32,2048,1024], W_w [1024,2048],
W_b [1024], v_w [1,1024], v_b [1] ->
  context [32,1024], weights [32,2048]

  energy  = tanh(hidden @ Wh^T + enc @ We^T + W_b)     (Wh = W_w[:, :H], We = W_w[:, H:])
  scores  = energy @ v_w[0]   (+ v_b, irrelevant to softmax)
  weights = softmax(scores, axis=seq)
  context = weights @ enc

Sharding: data-parallel over batch B across the 8 cores (4 batches/core),
full W/v replicated per core. No cross-core communication.

Per-core dataflow (all matmuls bf16 operands, fp32 PSUM accumulation):
  - W_w / hidden / v cast fp32->bf16 inline during SWDGE DMA (gpsimd).
  - W^T / hidden^T / enc^T produced by xbar DMA-transpose (2-byte dtype);
    a [128, n*128] -> [128, n, 128] transpose yields chunk layout
    out[p, c, j] = in[j, c*128 + p], i.e. natural 128-chunking of the
    contracted dimension onto partitions.
  - bias(b, o) = h_proj(b, o) + W_b(o) is fused into the tanh as the
    ScalarE activation per-partition bias (energy laid out [o, s]).
  - scores = sum_o v[o] * energy[o, s] via PSUM-accumulated matmuls.
  - softmax on a single partition row [1, 2048].
  - probs transposed (xbar) to [128, 16] so context = probs^T-weighted
    sum over s runs as PSUM-accumulated matmuls against the resident
    natural-layout bf16 enc tiles.
"""

import numpy as np
from contextlib import ExitStack

import concourse.bass as bass
import concourse.mybir as mybir
import concourse.tile as tile
from concourse import bacc
from concourse.bass_utils import run_bass_kernel_spmd

B, S, H = 32, 2048, 1024
NCORES = 8
BL = B // NCORES          # batches per core
HC = H // 128             # h-chunks (contraction) = 8
OC = H // 128             # o-chunks (output feature) = 8
SC = S // 128             # s-chunks per batch = 16
ST = 512                  # matmul moving free-dim tile over s
NST = S // ST             # s-tiles per batch = 4

F32 = mybir.dt.float32
BF16 = mybir.dt.bfloat16
AF = mybir.ActivationFunctionType


def _body(ctx: ExitStack, tc: tile.TileContext, hidden_d, enc_d, ww_d, wb_d,
          vw_d, ctx_d, wts_d):
    nc = tc.nc

    singles = ctx.enter_context(tc.tile_pool(name="singles", bufs=1))

    # ---------------- weight / bias prep (once per core) ----------------
    whT = singles.tile([128, HC, H], BF16)   # whT[p,c,o] = Wh[o, c*128+p]
    weT = singles.tile([128, HC, H], BF16)   # weT[p,c,o] = We[o, c*128+p]
    wb_sb = singles.tile([128, HC], F32)     # wb_sb[p,c] = W_b[c*128+p]
    v16 = singles.tile([128, HC], BF16)      # v16[p,c]  = v_w[0, c*128+p]
    bias_sb = singles.tile([128, OC, 16], F32)  # bias_sb[p,oc,b] = hproj+W_b

    with tc.tile_pool(name="wprep", bufs=1) as wprep, \
         tc.tile_pool(name="hprep_ps", bufs=4, space="PSUM") as hps:
        w16 = wprep.tile([128, OC, 2 * H], BF16)  # w16[p,oc,c] = W_w[oc*128+p, c]
        nc.gpsimd.dma_start(out=w16, in_=ww_d.rearrange("(oc p) c -> p oc c", p=128))
        for oc in range(OC):
            nc.sync.dma_start(out=whT[:, :, oc * 128:(oc + 1) * 128],
                              in_=w16[:, oc, 0:H], transpose=True)
            nc.sync.dma_start(out=weT[:, :, oc * 128:(oc + 1) * 128],
                              in_=w16[:, oc, H:2 * H], transpose=True)

        nc.sync.dma_start(out=wb_sb, in_=wb_d.rearrange("(c p) -> p c", p=128))
        nc.gpsimd.dma_start(out=v16, in_=vw_d.rearrange("a (c p) -> p (a c)", p=128))

        hid16 = wprep.tile([16, H], BF16)
        nc.vector.memset(hid16, 0.0)
        nc.gpsimd.dma_start(out=hid16[0:BL, :], in_=hidden_d)
        hidT = wprep.tile([128, HC, 16], BF16)  # hidT[p,c,b] = hidden[b, c*128+p]
        nc.sync.dma_start(out=hidT, in_=hid16, transpose=True)

        # h_proj[o, b] = sum_h Wh[o, h] * hidden[b, h]; bias = h_proj + W_b
        for oc in range(OC):
            hp = hps.tile([128, 16], F32)
            for c in range(HC):
                nc.tensor.matmul(hp, lhsT=whT[:, c, oc * 128:(oc + 1) * 128],
                                 rhs=hidT[:, c, :],
                                 start=(c == 0), stop=(c == HC - 1))
            nc.vector.tensor_scalar_add(out=bias_sb[:, oc, :], in0=hp,
                                        scalar1=wb_sb[:, oc:oc + 1])

    # ---------------- main loop over local batches ----------------
    enc_pool = ctx.enter_context(tc.tile_pool(name="enc_nat", bufs=2))
    encT_pool = ctx.enter_context(tc.tile_pool(name="encT", bufs=2))
    energy_pool = ctx.enter_context(tc.tile_pool(name="energy", bufs=3))
    sm_pool = ctx.enter_context(tc.tile_pool(name="sm", bufs=2))
    p16_pool = ctx.enter_context(tc.tile_pool(name="p16", bufs=2))
    out_pool = ctx.enter_context(tc.tile_pool(name="outs", bufs=2))
    eproj_ps = ctx.enter_context(tc.tile_pool(name="eproj_ps", bufs=4, space="PSUM"))
    scores_ps = ctx.enter_context(tc.tile_pool(name="scores_ps", bufs=2, space="PSUM"))
    ctx_ps = ctx.enter_context(tc.tile_pool(name="ctx_ps", bufs=1, space="PSUM"))

    enc_r = enc_d.rearrange("b (sc p) h -> b p sc h", p=128)

    for b in range(BL):
        # cast-load enc natural layout: enc_nat[p, sc, h] = enc[b, sc*128+p, h]
        enc_nat = enc_pool.tile([128, SC, H], BF16)
        for q in range(4):
            nc.gpsimd.dma_start(out=enc_nat[:, q * 4:(q + 1) * 4, :],
                                in_=enc_r[b, :, q * 4:(q + 1) * 4, :])

        scores_sb = sm_pool.tile([1, S], F32)

        for half in range(2):
            # transpose half a batch: encT[p, c, s] = enc[b, s0+s, c*128+p]
            encT = encT_pool.tile([128, HC, S // 2], BF16)
            for scl in range(SC // 2):
                sc = half * (SC // 2) + scl
                nc.sync.dma_start(out=encT[:, :, scl * 128:(scl + 1) * 128],
                                  in_=enc_nat[:, sc, :], transpose=True)

            for stl in range(NST // 2):
                st = half * (NST // 2) + stl
                s0 = stl * ST
                sc_tile = scores_ps.tile([1, ST], F32)
                for oc in range(OC):
                    ep = eproj_ps.tile([128, ST], F32)
                    for c in range(HC):
                        nc.tensor.matmul(ep,
                                         lhsT=weT[:, c, oc * 128:(oc + 1) * 128],
                                         rhs=encT[:, c, s0:s0 + ST],
                                         start=(c == 0), stop=(c == HC - 1))
                    en = energy_pool.tile([128, ST], BF16)
                    nc.scalar.activation(out=en, in_=ep, func=AF.Tanh,
                                         bias=bias_sb[:, oc, b:b + 1])
                    nc.tensor.matmul(sc_tile, lhsT=v16[:, oc:oc + 1], rhs=en,
                                     start=(oc == 0), stop=(oc == OC - 1))
                nc.scalar.copy(out=scores_sb[0:1, st * ST:(st + 1) * ST],
                               in_=sc_tile)

        # softmax over the full row [1, S] (single partition)
        neg_m = sm_pool.tile([1, 1], F32)
        nc.vector.reduce_max(out=neg_m, in_=scores_sb, axis=mybir.AxisListType.X,
                             negate=True)
        probs_f = sm_pool.tile([1, S], F32)
        nc.scalar.activation(out=probs_f, in_=scores_sb, func=AF.Exp, bias=neg_m)
        ssum = sm_pool.tile([1, 1], F32)
        nc.vector.reduce_sum(out=ssum, in_=probs_f, axis=mybir.AxisListType.X)
        rinv = sm_pool.tile([1, 1], F32)
        nc.vector.reciprocal(out=rinv, in_=ssum)
        wts_f = out_pool.tile([1, S], F32)
        nc.vector.tensor_scalar_mul(out=wts_f, in0=probs_f, scalar1=rinv)
        nc.sync.dma_start(out=wts_d[b:b + 1, :], in_=wts_f)

        # transpose normalized weights to [128, 16] for the context matmul
        probs16 = p16_pool.tile([16, S], BF16)
        nc.vector.memset(probs16, 0.0)
        nc.vector.tensor_copy(out=probs16[0:1, :], in_=wts_f)
        probsT = p16_pool.tile([128, SC, 16], BF16)  # probsT[p,c,0] = w[c*128+p]
        nc.sync.dma_start(out=probsT, in_=probs16, transpose=True)

        ctxp = ctx_ps.tile([1, H], F32)
        for c in range(SC):
            for h2 in range(2):
                nc.tensor.matmul(ctxp[0:1, h2 * 512:(h2 + 1) * 512],
                                 lhsT=probsT[:, c, 0:1],
                                 rhs=enc_nat[:, c, h2 * 512:(h2 + 1) * 512],
                                 start=(c == 0), stop=(c == SC - 1))
        ctx_sb = out_pool.tile([1, H], F32)
        nc.scalar.copy(out=ctx_sb, in_=ctxp)
        nc.sync.dma_start(out=ctx_d[b:b + 1, :], in_=ctx_sb)


def build():
    nc = bacc.Bacc("TRN2", target_bir_lowering=False, debug=False,
                   enable_asserts=False, num_devices=NCORES)
    hidden_d = nc.dram_tensor("hidden", [BL, H], F32, kind="ExternalInput").ap()
    enc_d = nc.dram_tensor("enc", [BL, S, H], F32, kind="ExternalInput").ap()
    ww_d = nc.dram_tensor("w_w", [H, 2 * H], F32, kind="ExternalInput").ap()
    wb_d = nc.dram_tensor("w_b", [H], F32, kind="ExternalInput").ap()
    vw_d = nc.dram_tensor("v_w", [1, H], F32, kind="ExternalInput").ap()
    ctx_d = nc.dram_tensor("ctx", [BL, H], F32, kind="ExternalOutput").ap()
    wts_d = nc.dram_tensor("wts", [BL, S], F32, kind="ExternalOutput").ap()

    with tile.TileContext(nc) as tc:
        with ExitStack() as stack:
            _body(stack, tc, hidden_d, enc_d, ww_d, wb_d, vw_d, ctx_d, wts_d)
    nc.compile()
    return nc


_CACHE: dict = {}


def get_nc():
    if "nc" not in _CACHE:
        _CACHE["nc"] = build()
    return _CACHE["nc"]


def make_in_maps(hidden, encoder_outputs, W_w, W_b, v_w):
    hidden = np.asarray(hidden, dtype=np.float32)
    enc = np.asarray(encoder_outputs, dtype=np.float32)
    ww = np.ascontiguousarray(np.asarray(W_w, dtype=np.float32))
    wb = np.ascontiguousarray(np.asarray(W_b, dtype=np.float32))
    vw = np.ascontiguousarray(np.asarray(v_w, dtype=np.float32))
    in_maps = []
    for core in range(NCORES):
        sl = slice(core * BL, (core + 1) * BL)
        in_maps.append({
            "hidden": np.ascontiguousarray(hidden[sl]),
            "enc": np.ascontiguousarray(enc[sl]),
            "w_w": ww,
            "w_b": wb,
            "v_w": vw,
        })
    return in_maps


def kernel(hidden, encoder_outputs, W_w, W_b, v_w, v_b):
    nc = get_nc()
    in_maps = make_in_maps(hidden, encoder_outputs, W_w, W_b, v_w)
    res = run_bass_kernel_spmd(nc, in_maps, core_ids=list(range(NCORES)))
    ctx = np.concatenate([res.results[c]["ctx"] for c in range(NCORES)], axis=0)
    wts = np.concatenate([res.results[c]["wts"] for c in range(NCORES)], axis=0)
    return ctx.astype(np.float32), wts.astype(np.float32)
